# revision 14
# baseline (speedup 1.0000x reference)
"""Trainium2 Bass kernel for nn_Activation1d (upsample2x-linear -> SiLU -> downsample2x).

Math: with align_corners=False linear resize, UP_RATIO=2, the whole op reduces
to a 3-tap stencil along T:
    a[j] = 0.25*(3*x[j] + x[j-1])      (x[-1] clamped to x[0])
    b[j] = 0.25*(3*x[j] + x[j+1])      (x[T] clamped to x[T-1])
    out[j] = 0.5*(silu(a[j]) + silu(b[j]))

Pure pointwise over (B, C): shard B*C = 8192 rows across 8 cores, T stays local.
"""

import os
import sys
from contextlib import ExitStack

import numpy as np

for _p in ("/opt/trn_rl_repo",):
    if _p not in sys.path:
        sys.path.insert(0, _p)

import bass_rust
import concourse.bass as bass
import concourse.mybir as mybir
from concourse import tile
from concourse.bass_utils import run_bass_kernel_spmd

N_CORES = 8
B, C, T = 16, 512, 8192
ROWS = B * C                 # 8192
RPC = ROWS // N_CORES        # 1024 rows per core
P = 128                      # SBUF partitions
N_RT = RPC // P              # 8 row-tiles per core
W = 2048                     # free-dim compute chunk width
NCH = T // W                 # chunks per row-tile

ALU = mybir.AluOpType
AFT = mybir.ActivationFunctionType
F32 = mybir.dt.float32

# --- tunables (env-overridable for experiments) ---
CDT_NAME = os.environ.get("K_CDT", "float16")       # compute dtype for DVE ops
USE_STT = os.environ.get("K_STT", "1") == "1"        # scalar_tensor_tensor vs (t3 + add)
# Inputs via SWDGE (gpsimd); outputs MUST be HWDGE (sync): compute
# instructions waiting on an SWDGE out-DMA's lane semaphore hang the device
# (the +16 never lands), while the same WAR pattern on DMAHW lanes works.
OUT_DMA_ENGINE = os.environ.get("K_ODMA", "sync")
IN_DMA_ENGINE = os.environ.get("K_IDMA", "gpsimd")

_LAST_EXEC_NS = None
_LAST_RESULT = None


def _build():
    cdt = getattr(mybir.dt, CDT_NAME)
    # Tile's stale SBUF cap (192K) leaves real capacity (208K usable) unused;
    # this kernel needs ~197K per partition.
    import concourse.tile_utils as _tu

    _tu.max_sbuf_usage = 208 * 1024
    nc = bass.Bass()
    # Input arrives host-padded to T+2 columns: x_pad[:, 0] = x[:, 0],
    # x_pad[:, 1:T+1] = x, x_pad[:, T+1] = x[:, T-1]. The clamp duplicates
    # are baked in on the host so no SBUF edge fills are needed and every
    # +-1-shifted stencil read is an aligned f32 view.
    x_ext = nc.declare_dram_parameter("x", [RPC, T + 2], F32, isOutput=False)
    o_ext = nc.declare_dram_parameter("out", [RPC, T], F32, isOutput=True)

    with tile.TileContext(nc) as tc:
        with ExitStack() as ctx:
            xpool = ctx.enter_context(tc.tile_pool(name="xp", bufs=2))
            tpool = ctx.enter_context(tc.tile_pool(name="tp", bufs=2))
            opool = ctx.enter_context(tc.tile_pool(name="op", bufs=2))

            in_dma = getattr(nc, IN_DMA_ENGINE)
            out_dma = getattr(nc, OUT_DMA_ENGINE)

            # [P,1] scalar-slot constants for the custom-DVE ops (the TTSS
            # scale slots want DATA_PTR APs; float immediates mis-encode).
            cpool = ctx.enter_context(tc.tile_pool(name="cp", bufs=1))
            c3 = cpool.tile([P, 1], F32, tag="c3")
            c0 = cpool.tile([P, 1], F32, tag="c0")
            scr = cpool.tile([P, 1], F32, tag="scr")
            nc.vector.memset(c3[:], 3.0)
            nc.vector.memset(c0[:], 0.0)

            # DMA budget: broken DGE lane-reuse in this stack means at most 8
            # DMAs per ring (SWDGE qPoolDynamic / HWDGE qSPDynamicHW) so no
            # lane is ever reused: 8 full-row loads (SWDGE) + 8 full-row
            # stores (sync HWDGE).
            #
            # Software pipeline, back half shifted by one tile: per iteration
            # the DVE stream is [absorb, ATAa(r), ATAb(r), claim, TT(r-1)] and
            # ACT is [silu_a(r), silu_b(r), mul(r-1)], so neither engine ever
            # waits mid-tile on the other's fresh output. Every instruction
            # carries at most one semaphore wait (walrus encodes only one).
            live = {}
            for r in range(N_RT + 1):
                if r < N_RT:
                    rows = slice(r * P, (r + 1) * P)
                    xt = xpool.tile([P, T + 2], F32, tag="xt")
                    in_dma.dma_start(xt[:], x_ext[rows, :])
                    # Tiny wait-absorber: takes the DMA-lane wait so ATAa's
                    # single wait slot is free for its ta WAR (ACT mul r-2).
                    nc.vector.tensor_copy(scr[:], xt[:, 0:1])
                    # ta = 3*x[j] + x[j-1], tb = 3*x[j] + x[j+1]; one
                    # custom-DVE op each (f32 in, f16 out, 1.5 cyc/el).
                    ta = tpool.tile([P, T], cdt, tag="ta")
                    tb = tpool.tile([P, T], cdt, tag="tb")
                    nc.vector.affine_then_add(
                        ta[:], xt[:, 1 : T + 1], xt[:, 0:T], c3[:], c0[:]
                    )
                    nc.vector.affine_then_add(
                        tb[:], xt[:, 1 : T + 1], xt[:, 2 : T + 2], c3[:], c0[:]
                    )
                    # Dummy first-writer claim of oc on DVE: absorbs the WAR
                    # wait on the out-DMA lane (r-2), so the ACT mul below
                    # carries only one DVE-sem wait.
                    oc = opool.tile([P, T], F32, tag="oc")
                    nc.vector.memset(oc[:, 0:1], 0.0)
                    # silu in place, 0.25 folded into ACT's input scale
                    nc.scalar.activation(ta[:], ta[:], AFT.Silu, scale=0.25)
                    nc.scalar.activation(tb[:], tb[:], AFT.Silu, scale=0.25)
                    live[r] = (ta, tb, oc)
                if r >= 1:
                    ta, tb, oc = live.pop(r - 1)
                    prows = slice((r - 1) * P, r * P)
                    # sum at the f16 2x DVE rate; *0.5 with the f32 upconvert
                    # on ACT (Copy-with-scale) keeps DVE/ACT balanced.
                    nc.vector.tensor_add(ta[:], ta[:], tb[:])
                    nc.scalar.mul(oc[:], ta[:], 0.5)
                    out_dma.dma_start(o_ext[prows, :], oc[:])
    return nc


_PRUNABLE = (
    bass_rust.InstDMACopy,
    bass_rust.InstTensorCopy,
    bass_rust.InstTensorTensor,
    bass_rust.InstTensorScalarPtr,
    bass_rust.InstActivation,
    bass_rust.InstCustomDveAnt,
)


def _transitive_prune_waits(nc):
    """Reduce every prunable instruction to at most one semaphore wait.

    This walrus build's engine/DMA ISA structs hold a single sync wait per
    instruction, but Tile's scheduler emits one wait per dependent proc
    because its vector clock is not transitively minimal across procs.

    Phase 1 simulates the emitted program (greedy topological execution over
    per-engine in-order streams), recording for every semaphore value the
    happens-before knowledge it implies and a global feasible order.
    Phase 2 drops waits implied by program order + remaining waits; if more
    than one wait survives, it strengthens one wait (raising its threshold
    to a value already reached earlier in the phase-1 order, so no cycle can
    form) until that single wait implies all the others.

    Soundness: engines complete instructions in stream order (DVE/ACT/SP);
    per-lane DMA updates land in issue order (Tile serializes lane reuse);
    Pool compute may complete out of order across Q7 cores, so no transitive
    knowledge is propagated through the Pool semaphore.
    """
    f = nc.m.functions[0]
    streams = {}
    for b in f.blocks:
        for inst in b.instructions:
            streams.setdefault(str(inst.engine), []).append(inst)

    def merge(dst, src):
        for s, v in src.items():
            if dst.get(s, 0) < v:
                dst[s] = v

    # ---- phase 1: simulate, collect logs ----
    sem_val = {}
    sem_log = {}        # sem -> list of (cum_value, knowledge, step)
    proc_know = {e: {} for e in streams}
    proc_self = {e: {} for e in streams}
    ptr = {e: 0 for e in streams}
    inst_info = {}      # id(inst) -> (base knowledge, step)
    step = 0

    def knowledge_of(sem, val, max_step=None):
        k = {sem: val}
        if sem.startswith("Pool"):
            return k
        for cum, kn, st in sem_log.get(sem, ()):
            if max_step is not None and st >= max_step:
                break
            merge(k, kn)
            if cum >= val:
                break
        return k

    def satisfied(w):
        v = sem_val.get(w.ant_name, 0)
        return v == w.wait_value if w.wait_mode == "sem-eq-imm" else v >= w.wait_value

    def execute(eng, inst):
        nonlocal step, done
        si = inst.sync_info
        waits = list(si.on_wait) if si is not None else []
        base = dict(proc_know[eng])
        merge(base, proc_self[eng])
        inst_info[id(inst)] = (dict(base), step)
        acc = base
        for w in waits:
            merge(acc, knowledge_of(w.ant_name, w.wait_value))
        proc_know[eng] = acc
        is_dma = isinstance(inst, bass_rust.InstDMACopy)
        if si is not None:
            for u in si.on_update:
                s = u.ant_name
                dv = {
                    "sem-add-imm": u.update_value,
                    "sem-inc": 1,
                    "sem-dec": -1,
                    "sem-sub-imm": -u.update_value,
                }[u.update_mode]
                nv = sem_val.get(s, 0) + dv
                sem_val[s] = nv
                kn = dict(proc_know[eng])
                merge(kn, proc_self[eng])
                if not is_dma and eng != "EngineType.Pool":
                    # Pool (8 Q7 cores) completes out of order: a later Pool
                    # instruction cannot assume earlier ones finished.
                    proc_self[eng][s] = max(proc_self[eng].get(s, 0), nv)
                kn[s] = nv
                sem_log.setdefault(s, []).append((nv, kn, step))
        ptr[eng] += 1
        done += 1
        step += 1

    total = sum(len(s) for s in streams.values())
    done, progress = 0, True
    while done < total and progress:
        progress = False
        # Execute DMAs as late as possible so compute events order before
        # them in the recorded feasible order (maximizes strengthening).
        for eng, stream in streams.items():
            while ptr[eng] < len(stream):
                inst = stream[ptr[eng]]
                si = inst.sync_info
                waits = list(si.on_wait) if si is not None else []
                if isinstance(inst, bass_rust.InstDMACopy):
                    break
                if not all(satisfied(w) for w in waits):
                    break
                execute(eng, inst)
                progress = True
        if progress:
            continue
        for eng, stream in streams.items():
            if ptr[eng] < len(stream):
                inst = stream[ptr[eng]]
                si = inst.sync_info
                waits = list(si.on_wait) if si is not None else []
                if isinstance(inst, bass_rust.InstDMACopy) and all(
                    satisfied(w) for w in waits
                ):
                    execute(eng, inst)
                    progress = True
                    break
    if done < total:
        import logging

        logging.warning(
            "_transitive_prune_waits: simulation stalled at %d/%d; "
            "no pruning applied",
            done,
            total,
        )
        return

    # ---- phase 2: prune / strengthen ----
    remaining_multi = []
    for eng, stream in streams.items():
        for inst in stream:
            si = inst.sync_info
            waits = list(si.on_wait) if si is not None else []
            if len(waits) < 2:
                continue
            if not isinstance(inst, _PRUNABLE) or any(
                w.wait_mode != "sem-ge-imm" for w in waits
            ):
                remaining_multi.append(inst)
                continue
            base, my_step = inst_info[id(inst)]

            def implied(k, ws):
                return all(k.get(w.ant_name, 0) >= w.wait_value for w in ws)

            # A DMA's wait on its own update lane (Tile's lane-reuse
            # throttle) is load-bearing for the DGE hardware beyond its
            # ordering semantics: dropping it wedges the device even when
            # the ordering is transitively guaranteed. Never touch those.
            own_lanes = set()
            if isinstance(inst, bass_rust.InstDMACopy) and si is not None:
                own_lanes = {u.ant_name for u in si.on_update}
            fixed = [w for w in waits if w.ant_name in own_lanes]
            # 1) drop waits implied by base + the other waits (greedy, all orders)
            import itertools

            best = None
            for order in itertools.permutations(range(len(waits))):
                a = dict(base)
                for w in fixed:
                    merge(a, knowledge_of(w.ant_name, w.wait_value))
                kp = [i for i in range(len(waits)) if waits[i] in fixed]
                for i in order:
                    w = waits[i]
                    if w in fixed:
                        continue
                    if a.get(w.ant_name, 0) >= w.wait_value:
                        continue
                    kp.append(i)
                    merge(a, knowledge_of(w.ant_name, w.wait_value))
                if best is None or len(kp) < len(best):
                    best = kp
                if len(kp) <= 1:
                    break
            kept = [waits[i] for i in sorted(best)]
            # 2) strengthen: find one sem whose (possibly later) value implies all
            if len(kept) > 1 and fixed:
                remaining_multi.append(inst)
                continue
            if len(kept) > 1:
                chosen = None
                cands = sorted(
                    {w.ant_name for w in waits},
                    key=lambda s: (s.startswith("DMA"), s),
                )
                for s in cands:
                    if s.startswith("Pool"):
                        continue
                    k = dict(base)
                    for cum, kn, st in sem_log.get(s, ()):
                        if st >= my_step:
                            break  # only events already ordered before us
                        merge(k, kn)
                        k[s] = max(k.get(s, 0), cum)
                        if implied(k, waits):
                            chosen = (s, cum)
                            break
                    if chosen:
                        break
                if chosen:
                    tmpl = next(w for w in waits if w.ant_name == chosen[0])
                    tmpl.wait_value = chosen[1]
                    kept = [tmpl]
                else:
                    remaining_multi.append(inst)
                    continue
            if len(kept) != len(waits) or any(
                k.wait_value != w.wait_value for k, w in zip(kept, waits)
            ):
                si.on_wait = kept
                inst.sync_info = si
    # ---- phase 3: non-prunable multi-wait instructions (the tail drain) ----
    # Reduce to the minimal wait subset via transitivity, keep one wait, and
    # move the rest onto zero-wait tail instructions (event semaphores) that
    # execute before NEFF completion. Sound: the conditions depend only on
    # DMAs issued in the main region, so no donor can deadlock, and every
    # stream must finish before the NEFF signals done.
    import itertools as _it

    unresolved = []
    if remaining_multi:
        last_dma_step = max(
            (inst_info[id(i)][1] for s in streams.values() for i in s
             if isinstance(i, bass_rust.InstDMACopy) and id(i) in inst_info),
            default=0,
        )
        donors = [
            i
            for s in streams.values()
            for i in s
            if isinstance(
                i, (bass_rust.InstEventSemaphore, bass_rust.InstDrain)
            )
            and i.sync_info is not None
            and not list(i.sync_info.on_wait)
            and inst_info.get(id(i), (None, -1))[1] > last_dma_step
        ]
        for inst in remaining_multi:
            si = inst.sync_info
            waits = list(si.on_wait)
            if any(w.wait_mode != "sem-ge-imm" for w in waits):
                unresolved.append(inst)
                continue
            base, _st = inst_info[id(inst)]
            best = None
            for r in range(1, len(waits) + 1):
                for combo in _it.combinations(range(len(waits)), r):
                    k = dict(base)
                    for i in combo:
                        merge(k, knowledge_of(waits[i].ant_name, waits[i].wait_value))
                    if all(k.get(w.ant_name, 0) >= w.wait_value for w in waits):
                        best = [waits[i] for i in combo]
                        break
                if best:
                    break
            if best is None:
                best = waits
            extra = best[1:]
            if len(extra) > len(donors):
                unresolved.append(inst)
                continue
            for w in extra:
                d = donors.pop()
                dsi = d.sync_info
                dsi.on_wait = [w]
                d.sync_info = dsi
            si.on_wait = best[:1]
            inst.sync_info = si
    if unresolved:
        import logging

        logging.warning(
            "_transitive_prune_waits: %d instructions still multi-wait: %s",
            len(unresolved),
            [i.name for i in unresolved[:10]],
        )


_NC = None


def _get_nc():
    global _NC
    if _NC is None:
        _NC = _build()
        _transitive_prune_waits(_NC)
        # Populate .instr bytes for InstISA subclasses (custom-DVE ops).
        # Raw Bass doesn't run this pass; without it the NEFF compiler sees
        # empty .instr -> "ISA wrong length". Must run AFTER wait pruning:
        # the encoder bakes sync_info in and asserts <=1 wait per InstISA.
        from concourse.library_overlay import lower_extended_insts

        lower_extended_insts(_NC)
    return _NC


def kernel(x):
    global _LAST_EXEC_NS, _LAST_RESULT
    x = np.asarray(x, dtype=np.float32)
    assert x.shape == (B, C, T), x.shape
    flat = x.reshape(ROWS, T)
    # Bake the stencil clamp duplicates in on the host: pad[:, m] = x[m-1]
    # with x[-1] := x[0] and x[T] := x[T-1].
    pad = np.empty((ROWS, T + 2), dtype=np.float32)
    pad[:, 1 : T + 1] = flat
    pad[:, 0] = flat[:, 0]
    pad[:, T + 1] = flat[:, T - 1]
    in_maps = [
        {"x": np.ascontiguousarray(pad[i * RPC : (i + 1) * RPC])}
        for i in range(N_CORES)
    ]
    nc = _get_nc()
    res = run_bass_kernel_spmd(
        nc,
        in_maps,
        core_ids=list(range(N_CORES)),
        trace=os.environ.get("K_TRACE", "0") == "1",
    )
    _LAST_RESULT = res
    _LAST_EXEC_NS = res.exec_time_ns
    out = np.concatenate([r["out"] for r in res.results], axis=0)
    return np.ascontiguousarray(out.reshape(B, C, T))



# revision 15
# speedup vs baseline: 1.0452x; 1.0452x over previous
"""Trainium2 Bass kernel for nn_Activation1d (upsample2x-linear -> SiLU -> downsample2x).

Math: with align_corners=False linear resize, UP_RATIO=2, the whole op reduces
to a 3-tap stencil along T:
    a[j] = 0.25*(3*x[j] + x[j-1])      (x[-1] clamped to x[0])
    b[j] = 0.25*(3*x[j] + x[j+1])      (x[T] clamped to x[T-1])
    out[j] = 0.5*(silu(a[j]) + silu(b[j]))

Pure pointwise over (B, C): shard B*C = 8192 rows across 8 cores, T stays local.
"""

import os
import sys
from contextlib import ExitStack

import numpy as np

for _p in ("/opt/trn_rl_repo",):
    if _p not in sys.path:
        sys.path.insert(0, _p)

import bass_rust
import concourse.bass as bass
import concourse.mybir as mybir
from concourse import tile
from concourse.bass_utils import run_bass_kernel_spmd

N_CORES = 8
B, C, T = 16, 512, 8192
ROWS = B * C                 # 8192
RPC = ROWS // N_CORES        # 1024 rows per core
P = 128                      # SBUF partitions
N_RT = RPC // P              # 8 row-tiles per core
W = 2048                     # free-dim compute chunk width
NCH = T // W                 # chunks per row-tile

ALU = mybir.AluOpType
AFT = mybir.ActivationFunctionType
F32 = mybir.dt.float32

# --- tunables (env-overridable for experiments) ---
CDT_NAME = os.environ.get("K_CDT", "float16")       # compute dtype for DVE ops
USE_STT = os.environ.get("K_STT", "1") == "1"        # scalar_tensor_tensor vs (t3 + add)
# Inputs via SWDGE (gpsimd); outputs MUST be HWDGE (sync): compute
# instructions waiting on an SWDGE out-DMA's lane semaphore hang the device
# (the +16 never lands), while the same WAR pattern on DMAHW lanes works.
OUT_DMA_ENGINE = os.environ.get("K_ODMA", "sync")
IN_DMA_ENGINE = os.environ.get("K_IDMA", "gpsimd")

_LAST_EXEC_NS = None
_LAST_RESULT = None


def _build():
    cdt = getattr(mybir.dt, CDT_NAME)
    # Tile's stale SBUF cap (192K) leaves real capacity (208K usable) unused;
    # this kernel needs ~197K per partition.
    import concourse.tile_utils as _tu

    _tu.max_sbuf_usage = 208 * 1024
    nc = bass.Bass()
    # Input arrives host-padded to T+2 columns: x_pad[:, 0] = x[:, 0],
    # x_pad[:, 1:T+1] = x, x_pad[:, T+1] = x[:, T-1]. The clamp duplicates
    # are baked in on the host so no SBUF edge fills are needed and every
    # +-1-shifted stencil read is an aligned f32 view.
    x_ext = nc.declare_dram_parameter("x", [RPC, T + 2], F32, isOutput=False)
    o_ext = nc.declare_dram_parameter("out", [RPC, T], F32, isOutput=True)

    # Back-half finisher assignment: LN tiles do oc = 0.5*(sa+sb) in one
    # custom-DVE op (8.8 us, all DVE); the rest split it as TT-add on DVE
    # (4.4 us) + Copy-with-scale on ACT (7.1 us). n=3 LN tiles balances
    # DVE ~= ACT ~= 148 us.
    ln_tiles = {1, 4, 6}

    with tile.TileContext(nc) as tc:
        with ExitStack() as ctx:
            xpool = ctx.enter_context(tc.tile_pool(name="xp", bufs=4))
            tpool = ctx.enter_context(tc.tile_pool(name="tp", bufs=2))
            opool = ctx.enter_context(tc.tile_pool(name="op", bufs=2))

            in_dma = getattr(nc, IN_DMA_ENGINE)
            out_dma = getattr(nc, OUT_DMA_ENGINE)

            # [P,1] scalar-slot constants for ln_bwd_dx (the TTSS scale
            # slots want DATA_PTR APs; float immediates mis-encode).
            cpool = ctx.enter_context(tc.tile_pool(name="cp", bufs=1))
            c0 = cpool.tile([P, 1], F32, tag="c0")
            cm1 = cpool.tile([P, 1], F32, tag="cm1")
            scr = cpool.tile([P, 2], cdt, tag="scr")
            nc.vector.memset(c0[:], 0.0)
            nc.vector.memset(cm1[:], -1.0)

            # DMA budget: broken DGE lane-reuse in this stack means at most 8
            # DMAs per ring (SWDGE qPoolDynamic / HWDGE qSPDynamicHW) so no
            # lane is ever reused: 8 full-row loads (SWDGE) + 8 full-row
            # stores (sync HWDGE). The in-DMA casts f32->f16 in flight
            # (SWDGE-only feature): SBUF holds only f16, so 4 xs bufs fit
            # and the ~25 us load latency comes off the critical path.
            #
            # Software pipeline, back half shifted one tile so neither
            # engine waits mid-tile on the other's fresh output; every
            # instruction carries at most one semaphore wait (walrus
            # encodes only one).
            live = {}
            for r in range(N_RT + 1):
                if r < N_RT:
                    rows = slice(r * P, (r + 1) * P)
                    xs = xpool.tile([P, T + 2], cdt, tag="xs")
                    in_dma.dma_start(xs[:], x_ext[rows, :])
                    # Tiny wait-absorber: takes the DMA-lane wait so the TS
                    # below has its single wait slot free for its ta WAR.
                    nc.vector.tensor_copy(scr[:], xs[:, 0:2])
                    # ta <- 3*x[j] (misaligned f16 read, works at line rate)
                    # tb <- 3*x[j] + x[j+1]; then ta += x[j-1] in place.
                    ta = tpool.tile([P, T], cdt, tag="ta")
                    tb = tpool.tile([P, T], cdt, tag="tb")
                    nc.vector.tensor_scalar_mul(ta[:], xs[:, 1 : T + 1], 3.0)
                    nc.vector.tensor_add(tb[:], ta[:], xs[:, 2 : T + 2])
                    nc.vector.tensor_add(ta[:], ta[:], xs[:, 0:T])
                    # Dummy first-writer claim of oc on DVE: absorbs the WAR
                    # wait on the out-DMA lane (r-2) so the finisher carries
                    # only its RAW wait.
                    oc = opool.tile([P, T], F32, tag="oc")
                    nc.vector.memset(oc[:, 0:1], 0.0)
                    # silu in place, 0.25 folded into ACT's input scale;
                    # silu_b first so it overlaps the in-place ta add.
                    nc.scalar.activation(tb[:], tb[:], AFT.Silu, scale=0.25)
                    nc.scalar.activation(ta[:], ta[:], AFT.Silu, scale=0.25)
                    live[r] = (ta, tb, oc)
                if r >= 1:
                    ta, tb, oc = live.pop(r - 1)
                    prows = slice((r - 1) * P, r * P)
                    if (r - 1) in ln_tiles:
                        # oc = (ta - tb*(-1) - 0)*0.5, f16 -> f32, one DVE op
                        nc.vector.ln_bwd_dx(
                            oc[:], ta[:], tb[:], cm1[:], c0[:], scale=0.5
                        )
                    else:
                        nc.vector.tensor_add(ta[:], ta[:], tb[:])
                        nc.scalar.mul(oc[:], ta[:], 0.5)
                    out_dma.dma_start(o_ext[prows, :], oc[:])
    return nc


_PRUNABLE = (
    bass_rust.InstDMACopy,
    bass_rust.InstTensorCopy,
    bass_rust.InstTensorTensor,
    bass_rust.InstTensorScalarPtr,
    bass_rust.InstActivation,
    bass_rust.InstCustomDveAnt,
)


def _transitive_prune_waits(nc):
    """Reduce every prunable instruction to at most one semaphore wait.

    This walrus build's engine/DMA ISA structs hold a single sync wait per
    instruction, but Tile's scheduler emits one wait per dependent proc
    because its vector clock is not transitively minimal across procs.

    Phase 1 simulates the emitted program (greedy topological execution over
    per-engine in-order streams), recording for every semaphore value the
    happens-before knowledge it implies and a global feasible order.
    Phase 2 drops waits implied by program order + remaining waits; if more
    than one wait survives, it strengthens one wait (raising its threshold
    to a value already reached earlier in the phase-1 order, so no cycle can
    form) until that single wait implies all the others.

    Soundness: engines complete instructions in stream order (DVE/ACT/SP);
    per-lane DMA updates land in issue order (Tile serializes lane reuse);
    Pool compute may complete out of order across Q7 cores, so no transitive
    knowledge is propagated through the Pool semaphore.
    """
    f = nc.m.functions[0]
    streams = {}
    for b in f.blocks:
        for inst in b.instructions:
            streams.setdefault(str(inst.engine), []).append(inst)

    def merge(dst, src):
        for s, v in src.items():
            if dst.get(s, 0) < v:
                dst[s] = v

    # ---- phase 1: simulate, collect logs ----
    sem_val = {}
    sem_log = {}        # sem -> list of (cum_value, knowledge, step)
    proc_know = {e: {} for e in streams}
    proc_self = {e: {} for e in streams}
    ptr = {e: 0 for e in streams}
    inst_info = {}      # id(inst) -> (base knowledge, step)
    step = 0

    def knowledge_of(sem, val, max_step=None):
        k = {sem: val}
        if sem.startswith("Pool"):
            return k
        for cum, kn, st in sem_log.get(sem, ()):
            if max_step is not None and st >= max_step:
                break
            merge(k, kn)
            if cum >= val:
                break
        return k

    def satisfied(w):
        v = sem_val.get(w.ant_name, 0)
        return v == w.wait_value if w.wait_mode == "sem-eq-imm" else v >= w.wait_value

    def execute(eng, inst):
        nonlocal step, done
        si = inst.sync_info
        waits = list(si.on_wait) if si is not None else []
        base = dict(proc_know[eng])
        merge(base, proc_self[eng])
        inst_info[id(inst)] = (dict(base), step)
        acc = base
        for w in waits:
            merge(acc, knowledge_of(w.ant_name, w.wait_value))
        proc_know[eng] = acc
        is_dma = isinstance(inst, bass_rust.InstDMACopy)
        if si is not None:
            for u in si.on_update:
                s = u.ant_name
                dv = {
                    "sem-add-imm": u.update_value,
                    "sem-inc": 1,
                    "sem-dec": -1,
                    "sem-sub-imm": -u.update_value,
                }[u.update_mode]
                nv = sem_val.get(s, 0) + dv
                sem_val[s] = nv
                kn = dict(proc_know[eng])
                merge(kn, proc_self[eng])
                if not is_dma and eng != "EngineType.Pool":
                    # Pool (8 Q7 cores) completes out of order: a later Pool
                    # instruction cannot assume earlier ones finished.
                    proc_self[eng][s] = max(proc_self[eng].get(s, 0), nv)
                kn[s] = nv
                sem_log.setdefault(s, []).append((nv, kn, step))
        ptr[eng] += 1
        done += 1
        step += 1

    total = sum(len(s) for s in streams.values())
    done, progress = 0, True
    while done < total and progress:
        progress = False
        # Execute DMAs as late as possible so compute events order before
        # them in the recorded feasible order (maximizes strengthening).
        for eng, stream in streams.items():
            while ptr[eng] < len(stream):
                inst = stream[ptr[eng]]
                si = inst.sync_info
                waits = list(si.on_wait) if si is not None else []
                if isinstance(inst, bass_rust.InstDMACopy):
                    break
                if not all(satisfied(w) for w in waits):
                    break
                execute(eng, inst)
                progress = True
        if progress:
            continue
        for eng, stream in streams.items():
            if ptr[eng] < len(stream):
                inst = stream[ptr[eng]]
                si = inst.sync_info
                waits = list(si.on_wait) if si is not None else []
                if isinstance(inst, bass_rust.InstDMACopy) and all(
                    satisfied(w) for w in waits
                ):
                    execute(eng, inst)
                    progress = True
                    break
    if done < total:
        import logging

        logging.warning(
            "_transitive_prune_waits: simulation stalled at %d/%d; "
            "no pruning applied",
            done,
            total,
        )
        return

    # ---- phase 2: prune / strengthen ----
    remaining_multi = []
    for eng, stream in streams.items():
        for inst in stream:
            si = inst.sync_info
            waits = list(si.on_wait) if si is not None else []
            if len(waits) < 2:
                continue
            if not isinstance(inst, _PRUNABLE) or any(
                w.wait_mode != "sem-ge-imm" for w in waits
            ):
                remaining_multi.append(inst)
                continue
            base, my_step = inst_info[id(inst)]

            def implied(k, ws):
                return all(k.get(w.ant_name, 0) >= w.wait_value for w in ws)

            # A DMA's wait on its own update lane (Tile's lane-reuse
            # throttle) is load-bearing for the DGE hardware beyond its
            # ordering semantics: dropping it wedges the device even when
            # the ordering is transitively guaranteed. Never touch those.
            own_lanes = set()
            if isinstance(inst, bass_rust.InstDMACopy) and si is not None:
                own_lanes = {u.ant_name for u in si.on_update}
            fixed = [w for w in waits if w.ant_name in own_lanes]
            # 1) drop waits implied by base + the other waits (greedy, all orders)
            import itertools

            best = None
            for order in itertools.permutations(range(len(waits))):
                a = dict(base)
                for w in fixed:
                    merge(a, knowledge_of(w.ant_name, w.wait_value))
                kp = [i for i in range(len(waits)) if waits[i] in fixed]
                for i in order:
                    w = waits[i]
                    if w in fixed:
                        continue
                    if a.get(w.ant_name, 0) >= w.wait_value:
                        continue
                    kp.append(i)
                    merge(a, knowledge_of(w.ant_name, w.wait_value))
                if best is None or len(kp) < len(best):
                    best = kp
                if len(kp) <= 1:
                    break
            kept = [waits[i] for i in sorted(best)]
            # 2) strengthen: find one sem whose (possibly later) value implies all
            if len(kept) > 1 and fixed:
                remaining_multi.append(inst)
                continue
            if len(kept) > 1:
                chosen = None
                cands = sorted(
                    {w.ant_name for w in waits},
                    key=lambda s: (s.startswith("DMA"), s),
                )
                for s in cands:
                    if s.startswith("Pool"):
                        continue
                    k = dict(base)
                    for cum, kn, st in sem_log.get(s, ()):
                        if st >= my_step:
                            break  # only events already ordered before us
                        merge(k, kn)
                        k[s] = max(k.get(s, 0), cum)
                        if implied(k, waits):
                            chosen = (s, cum)
                            break
                    if chosen:
                        break
                if chosen:
                    tmpl = next(w for w in waits if w.ant_name == chosen[0])
                    tmpl.wait_value = chosen[1]
                    kept = [tmpl]
                else:
                    remaining_multi.append(inst)
                    continue
            if len(kept) != len(waits) or any(
                k.wait_value != w.wait_value for k, w in zip(kept, waits)
            ):
                si.on_wait = kept
                inst.sync_info = si
    # ---- phase 3: non-prunable multi-wait instructions (the tail drain) ----
    # Reduce to the minimal wait subset via transitivity, keep one wait, and
    # move the rest onto zero-wait tail instructions (event semaphores) that
    # execute before NEFF completion. Sound: the conditions depend only on
    # DMAs issued in the main region, so no donor can deadlock, and every
    # stream must finish before the NEFF signals done.
    import itertools as _it

    unresolved = []
    if remaining_multi:
        last_dma_step = max(
            (inst_info[id(i)][1] for s in streams.values() for i in s
             if isinstance(i, bass_rust.InstDMACopy) and id(i) in inst_info),
            default=0,
        )
        donors = [
            i
            for s in streams.values()
            for i in s
            if isinstance(
                i, (bass_rust.InstEventSemaphore, bass_rust.InstDrain)
            )
            and i.sync_info is not None
            and not list(i.sync_info.on_wait)
            and inst_info.get(id(i), (None, -1))[1] > last_dma_step
        ]
        for inst in remaining_multi:
            si = inst.sync_info
            waits = list(si.on_wait)
            if any(w.wait_mode != "sem-ge-imm" for w in waits):
                unresolved.append(inst)
                continue
            base, _st = inst_info[id(inst)]
            best = None
            for r in range(1, len(waits) + 1):
                for combo in _it.combinations(range(len(waits)), r):
                    k = dict(base)
                    for i in combo:
                        merge(k, knowledge_of(waits[i].ant_name, waits[i].wait_value))
                    if all(k.get(w.ant_name, 0) >= w.wait_value for w in waits):
                        best = [waits[i] for i in combo]
                        break
                if best:
                    break
            if best is None:
                best = waits
            extra = best[1:]
            if len(extra) > len(donors):
                unresolved.append(inst)
                continue
            for w in extra:
                d = donors.pop()
                dsi = d.sync_info
                dsi.on_wait = [w]
                d.sync_info = dsi
            si.on_wait = best[:1]
            inst.sync_info = si
    if unresolved:
        import logging

        logging.warning(
            "_transitive_prune_waits: %d instructions still multi-wait: %s",
            len(unresolved),
            [i.name for i in unresolved[:10]],
        )


_NC = None


def _get_nc():
    global _NC
    if _NC is None:
        _NC = _build()
        _transitive_prune_waits(_NC)
        # Populate .instr bytes for InstISA subclasses (custom-DVE ops).
        # Raw Bass doesn't run this pass; without it the NEFF compiler sees
        # empty .instr -> "ISA wrong length". Must run AFTER wait pruning:
        # the encoder bakes sync_info in and asserts <=1 wait per InstISA.
        from concourse.library_overlay import lower_extended_insts

        lower_extended_insts(_NC)
    return _NC


def kernel(x):
    global _LAST_EXEC_NS, _LAST_RESULT
    x = np.asarray(x, dtype=np.float32)
    assert x.shape == (B, C, T), x.shape
    flat = x.reshape(ROWS, T)
    # Bake the stencil clamp duplicates in on the host: pad[:, m] = x[m-1]
    # with x[-1] := x[0] and x[T] := x[T-1].
    pad = np.empty((ROWS, T + 2), dtype=np.float32)
    pad[:, 1 : T + 1] = flat
    pad[:, 0] = flat[:, 0]
    pad[:, T + 1] = flat[:, T - 1]
    in_maps = [
        {"x": np.ascontiguousarray(pad[i * RPC : (i + 1) * RPC])}
        for i in range(N_CORES)
    ]
    nc = _get_nc()
    res = run_bass_kernel_spmd(
        nc,
        in_maps,
        core_ids=list(range(N_CORES)),
        trace=os.environ.get("K_TRACE", "0") == "1",
    )
    _LAST_RESULT = res
    _LAST_EXEC_NS = res.exec_time_ns
    out = np.concatenate([r["out"] for r in res.results], axis=0)
    return np.ascontiguousarray(out.reshape(B, C, T))



# revision 18
# speedup vs baseline: 1.1489x; 1.0992x over previous
"""Trainium2 Bass kernel for nn_Activation1d (upsample2x-linear -> SiLU -> downsample2x).

Math: with align_corners=False linear resize, UP_RATIO=2, the whole op reduces
to a 3-tap stencil along T:
    a[j] = 0.25*(3*x[j] + x[j-1])      (x[-1] clamped to x[0])
    b[j] = 0.25*(3*x[j] + x[j+1])      (x[T] clamped to x[T-1])
    out[j] = 0.5*(silu(a[j]) + silu(b[j]))

Pure pointwise over (B, C): shard B*C = 8192 rows across 8 cores, T stays local.
"""

import os
import sys
from contextlib import ExitStack

import numpy as np

for _p in ("/opt/trn_rl_repo",):
    if _p not in sys.path:
        sys.path.insert(0, _p)

import bass_rust
import concourse.bass as bass
import concourse.mybir as mybir
from concourse import tile
from concourse.bass_utils import run_bass_kernel_spmd

N_CORES = 8
B, C, T = 16, 512, 8192
ROWS = B * C                 # 8192
RPC = ROWS // N_CORES        # 1024 rows per core
P = 128                      # SBUF partitions
N_RT = RPC // P              # 8 row-tiles per core
W = 2048                     # free-dim compute chunk width
NCH = T // W                 # chunks per row-tile

ALU = mybir.AluOpType
AFT = mybir.ActivationFunctionType
F32 = mybir.dt.float32

# --- tunables (env-overridable for experiments) ---
CDT_NAME = os.environ.get("K_CDT", "float16")       # compute dtype for DVE ops
USE_STT = os.environ.get("K_STT", "1") == "1"        # scalar_tensor_tensor vs (t3 + add)
# Inputs via SWDGE (gpsimd); outputs MUST be HWDGE (sync): compute
# instructions waiting on an SWDGE out-DMA's lane semaphore hang the device
# (the +16 never lands), while the same WAR pattern on DMAHW lanes works.
OUT_DMA_ENGINE = os.environ.get("K_ODMA", "sync")
IN_DMA_ENGINE = os.environ.get("K_IDMA", "gpsimd")

_LAST_EXEC_NS = None
_LAST_RESULT = None


def _build():
    cdt = getattr(mybir.dt, CDT_NAME)
    # Tile's stale SBUF cap (192K) leaves real capacity (208K usable) unused;
    # this kernel needs ~197K per partition.
    import concourse.tile_utils as _tu

    _tu.max_sbuf_usage = 208 * 1024
    nc = bass.Bass()
    # Input arrives host-padded to T+2 columns: x_pad[:, 0] = x[:, 0],
    # x_pad[:, 1:T+1] = x, x_pad[:, T+1] = x[:, T-1]. The clamp duplicates
    # are baked in on the host so no SBUF edge fills are needed and every
    # +-1-shifted stencil read is an aligned f32 view.
    x_ext = nc.declare_dram_parameter("x", [RPC, T + 2], F32, isOutput=False)
    o_ext = nc.declare_dram_parameter("out", [RPC, T], F32, isOutput=True)

    # Back-half finisher assignment: LN tiles do oc = 0.5*(sa+sb) in one
    # custom-DVE op (8.8 us, all DVE); mul tiles split it as TT-add on DVE
    # (4.4 us, into tb so the ACT WAR lands on TTb's free wait slot) +
    # Copy-with-scale on ACT (7.1 us). 4/4 balances DVE ~= ACT ~= 142 us,
    # under the ~160 us DMA floor.
    ln_tiles = {0, 2, 4, 6}

    with tile.TileContext(nc) as tc:
        with ExitStack() as ctx:
            xpool = ctx.enter_context(tc.tile_pool(name="xp", bufs=4))
            tpool = ctx.enter_context(tc.tile_pool(name="tp", bufs=2))
            opool = ctx.enter_context(tc.tile_pool(name="op", bufs=2))

            in_dma = getattr(nc, IN_DMA_ENGINE)
            out_dma = getattr(nc, OUT_DMA_ENGINE)

            # [P,1] scalar-slot constants for ln_bwd_dx (the TTSS scale
            # slots want DATA_PTR APs; float immediates mis-encode).
            cpool = ctx.enter_context(tc.tile_pool(name="cp", bufs=1))
            c0 = cpool.tile([P, 1], F32, tag="c0")
            cm1 = cpool.tile([P, 1], F32, tag="cm1")
            nc.vector.memset(c0[:], 0.0)
            nc.vector.memset(cm1[:], -1.0)

            # DMA budget: broken DGE lane-reuse in this stack means at most 8
            # DMAs per ring (SWDGE qPoolDynamic / HWDGE qSPDynamicHW) so no
            # lane is ever reused: 8 full-row loads (SWDGE) + 8 full-row
            # stores (sync HWDGE). The in-DMA casts f32->f16 in flight
            # (SWDGE-only feature): SBUF holds only f16, so 4 xs bufs fit
            # and the ~25 us load latency comes off the critical path.
            #
            # Software pipeline, back half shifted one tile so neither
            # engine waits mid-tile on the other's fresh output; every
            # instruction carries at most one semaphore wait (walrus
            # encodes only one).
            live = {}
            for r in range(N_RT + 1):
                if r < N_RT:
                    rows = slice(r * P, (r + 1) * P)
                    xs = xpool.tile([P, T + 2], cdt, tag="xs")
                    in_dma.dma_start(xs[:], x_ext[rows, :])
                    # ta <- 3*x[j] (misaligned f16 read runs in 2x mode);
                    # tb <- 3*x[j] + x[j+1]; then ta += x[j-1] in place.
                    # TS carries the DMA-lane wait (its ta WAR is program-
                    # order via the DVE finisher two tiles back); TTb carries
                    # the tb WAR (ACT mul r-2) in its free wait slot.
                    ta = tpool.tile([P, T], cdt, tag="ta")
                    tb = tpool.tile([P, T], cdt, tag="tb")
                    nc.vector.tensor_scalar_mul(ta[:], xs[:, 1 : T + 1], 3.0)
                    nc.vector.tensor_add(tb[:], ta[:], xs[:, 2 : T + 2])
                    nc.vector.tensor_add(ta[:], ta[:], xs[:, 0:T])
                    # Dummy first-writer claim of oc on DVE: absorbs the WAR
                    # wait on the out-DMA lane (r-2) so the finisher carries
                    # only its RAW wait.
                    oc = opool.tile([P, T], F32, tag="oc")
                    nc.vector.memset(oc[:, 0:1], 0.0)
                    # silu in place, 0.25 folded into ACT's input scale;
                    # silu_b first so it overlaps the in-place ta add.
                    nc.scalar.activation(tb[:], tb[:], AFT.Silu, scale=0.25)
                    nc.scalar.activation(ta[:], ta[:], AFT.Silu, scale=0.25)
                    live[r] = (ta, tb, oc)
                if r >= 1:
                    ta, tb, oc = live.pop(r - 1)
                    prows = slice((r - 1) * P, r * P)
                    if (r - 1) in ln_tiles:
                        # oc = (ta - tb*(-1) - 0)*0.5, f16 -> f32, one DVE op
                        nc.vector.ln_bwd_dx(
                            oc[:], ta[:], tb[:], cm1[:], c0[:], scale=0.5
                        )
                        out_dma.dma_start(o_ext[prows, :], oc[:])
                    else:
                        # sum into tb so ta's last reader stays on DVE and
                        # tb's WAR lands on TTb (which has a free wait slot)
                        nc.vector.tensor_add(tb[:], ta[:], tb[:])
                        nc.scalar.mul(oc[:], tb[:], 0.5)
                        out_dma.dma_start(o_ext[prows, :], oc[:])
    return nc


_PRUNABLE = (
    bass_rust.InstDMACopy,
    bass_rust.InstTensorCopy,
    bass_rust.InstTensorTensor,
    bass_rust.InstTensorScalarPtr,
    bass_rust.InstActivation,
    bass_rust.InstCustomDveAnt,
)


def _transitive_prune_waits(nc):
    """Reduce every prunable instruction to at most one semaphore wait.

    This walrus build's engine/DMA ISA structs hold a single sync wait per
    instruction, but Tile's scheduler emits one wait per dependent proc
    because its vector clock is not transitively minimal across procs.

    Phase 1 simulates the emitted program (greedy topological execution over
    per-engine in-order streams), recording for every semaphore value the
    happens-before knowledge it implies and a global feasible order.
    Phase 2 drops waits implied by program order + remaining waits; if more
    than one wait survives, it strengthens one wait (raising its threshold
    to a value already reached earlier in the phase-1 order, so no cycle can
    form) until that single wait implies all the others.

    Soundness: engines complete instructions in stream order (DVE/ACT/SP);
    per-lane DMA updates land in issue order (Tile serializes lane reuse);
    Pool compute may complete out of order across Q7 cores, so no transitive
    knowledge is propagated through the Pool semaphore.
    """
    f = nc.m.functions[0]
    streams = {}
    for b in f.blocks:
        for inst in b.instructions:
            streams.setdefault(str(inst.engine), []).append(inst)

    def merge(dst, src):
        for s, v in src.items():
            if dst.get(s, 0) < v:
                dst[s] = v

    # ---- phase 1: simulate, collect logs ----
    sem_val = {}
    sem_log = {}        # sem -> list of (cum_value, knowledge, step)
    proc_know = {e: {} for e in streams}
    proc_self = {e: {} for e in streams}
    ptr = {e: 0 for e in streams}
    inst_info = {}      # id(inst) -> (base knowledge, step)
    step = 0

    def knowledge_of(sem, val, max_step=None):
        k = {sem: val}
        if sem.startswith("Pool"):
            return k
        for cum, kn, st in sem_log.get(sem, ()):
            if max_step is not None and st >= max_step:
                break
            merge(k, kn)
            if cum >= val:
                break
        return k

    def satisfied(w):
        v = sem_val.get(w.ant_name, 0)
        return v == w.wait_value if w.wait_mode == "sem-eq-imm" else v >= w.wait_value

    def execute(eng, inst):
        nonlocal step, done
        si = inst.sync_info
        waits = list(si.on_wait) if si is not None else []
        base = dict(proc_know[eng])
        merge(base, proc_self[eng])
        inst_info[id(inst)] = (dict(base), step)
        acc = base
        for w in waits:
            merge(acc, knowledge_of(w.ant_name, w.wait_value))
        proc_know[eng] = acc
        is_dma = isinstance(inst, bass_rust.InstDMACopy)
        if si is not None:
            for u in si.on_update:
                s = u.ant_name
                dv = {
                    "sem-add-imm": u.update_value,
                    "sem-inc": 1,
                    "sem-dec": -1,
                    "sem-sub-imm": -u.update_value,
                }[u.update_mode]
                nv = sem_val.get(s, 0) + dv
                sem_val[s] = nv
                kn = dict(proc_know[eng])
                merge(kn, proc_self[eng])
                if not is_dma and eng != "EngineType.Pool":
                    # Pool (8 Q7 cores) completes out of order: a later Pool
                    # instruction cannot assume earlier ones finished.
                    proc_self[eng][s] = max(proc_self[eng].get(s, 0), nv)
                kn[s] = nv
                sem_log.setdefault(s, []).append((nv, kn, step))
        ptr[eng] += 1
        done += 1
        step += 1

    total = sum(len(s) for s in streams.values())
    done, progress = 0, True
    while done < total and progress:
        progress = False
        # Execute DMAs as late as possible so compute events order before
        # them in the recorded feasible order (maximizes strengthening).
        for eng, stream in streams.items():
            while ptr[eng] < len(stream):
                inst = stream[ptr[eng]]
                si = inst.sync_info
                waits = list(si.on_wait) if si is not None else []
                if isinstance(inst, bass_rust.InstDMACopy):
                    break
                if not all(satisfied(w) for w in waits):
                    break
                execute(eng, inst)
                progress = True
        if progress:
            continue
        for eng, stream in streams.items():
            if ptr[eng] < len(stream):
                inst = stream[ptr[eng]]
                si = inst.sync_info
                waits = list(si.on_wait) if si is not None else []
                if isinstance(inst, bass_rust.InstDMACopy) and all(
                    satisfied(w) for w in waits
                ):
                    execute(eng, inst)
                    progress = True
                    break
    if done < total:
        import logging

        logging.warning(
            "_transitive_prune_waits: simulation stalled at %d/%d; "
            "no pruning applied",
            done,
            total,
        )
        return

    # ---- phase 2: prune / strengthen ----
    remaining_multi = []
    for eng, stream in streams.items():
        for inst in stream:
            si = inst.sync_info
            waits = list(si.on_wait) if si is not None else []
            if len(waits) < 2:
                continue
            if not isinstance(inst, _PRUNABLE) or any(
                w.wait_mode != "sem-ge-imm" for w in waits
            ):
                remaining_multi.append(inst)
                continue
            base, my_step = inst_info[id(inst)]

            def implied(k, ws):
                return all(k.get(w.ant_name, 0) >= w.wait_value for w in ws)

            # A DMA's wait on its own update lane (Tile's lane-reuse
            # throttle) is load-bearing for the DGE hardware beyond its
            # ordering semantics: dropping it wedges the device even when
            # the ordering is transitively guaranteed. Never touch those.
            own_lanes = set()
            if isinstance(inst, bass_rust.InstDMACopy) and si is not None:
                own_lanes = {u.ant_name for u in si.on_update}
            fixed = [w for w in waits if w.ant_name in own_lanes]
            # 1) drop waits implied by base + the other waits (greedy, all orders)
            import itertools

            best = None
            for order in itertools.permutations(range(len(waits))):
                a = dict(base)
                for w in fixed:
                    merge(a, knowledge_of(w.ant_name, w.wait_value))
                kp = [i for i in range(len(waits)) if waits[i] in fixed]
                for i in order:
                    w = waits[i]
                    if w in fixed:
                        continue
                    if a.get(w.ant_name, 0) >= w.wait_value:
                        continue
                    kp.append(i)
                    merge(a, knowledge_of(w.ant_name, w.wait_value))
                if best is None or len(kp) < len(best):
                    best = kp
                if len(kp) <= 1:
                    break
            kept = [waits[i] for i in sorted(best)]
            # 2) strengthen: find one sem whose (possibly later) value implies all
            if len(kept) > 1 and fixed:
                remaining_multi.append(inst)
                continue
            if len(kept) > 1:
                chosen = None
                cands = sorted(
                    {w.ant_name for w in waits},
                    key=lambda s: (s.startswith("DMA"), s),
                )
                for s in cands:
                    if s.startswith("Pool"):
                        continue
                    k = dict(base)
                    for cum, kn, st in sem_log.get(s, ()):
                        if st >= my_step:
                            break  # only events already ordered before us
                        merge(k, kn)
                        k[s] = max(k.get(s, 0), cum)
                        if implied(k, waits):
                            chosen = (s, cum)
                            break
                    if chosen:
                        break
                if chosen:
                    tmpl = next(w for w in waits if w.ant_name == chosen[0])
                    tmpl.wait_value = chosen[1]
                    kept = [tmpl]
                else:
                    remaining_multi.append(inst)
                    continue
            if len(kept) != len(waits) or any(
                k.wait_value != w.wait_value for k, w in zip(kept, waits)
            ):
                si.on_wait = kept
                inst.sync_info = si
    # ---- phase 3: non-prunable multi-wait instructions (the tail drain) ----
    # Reduce to the minimal wait subset via transitivity, keep one wait, and
    # move the rest onto zero-wait tail instructions (event semaphores) that
    # execute before NEFF completion. Sound: the conditions depend only on
    # DMAs issued in the main region, so no donor can deadlock, and every
    # stream must finish before the NEFF signals done.
    import itertools as _it

    unresolved = []
    if remaining_multi:
        last_dma_step = max(
            (inst_info[id(i)][1] for s in streams.values() for i in s
             if isinstance(i, bass_rust.InstDMACopy) and id(i) in inst_info),
            default=0,
        )
        donors = [
            i
            for s in streams.values()
            for i in s
            if isinstance(
                i, (bass_rust.InstEventSemaphore, bass_rust.InstDrain)
            )
            and i.sync_info is not None
            and not list(i.sync_info.on_wait)
            and inst_info.get(id(i), (None, -1))[1] > last_dma_step
        ]
        for inst in remaining_multi:
            si = inst.sync_info
            waits = list(si.on_wait)
            if any(w.wait_mode != "sem-ge-imm" for w in waits):
                unresolved.append(inst)
                continue
            base, _st = inst_info[id(inst)]
            best = None
            for r in range(1, len(waits) + 1):
                for combo in _it.combinations(range(len(waits)), r):
                    k = dict(base)
                    for i in combo:
                        merge(k, knowledge_of(waits[i].ant_name, waits[i].wait_value))
                    if all(k.get(w.ant_name, 0) >= w.wait_value for w in waits):
                        best = [waits[i] for i in combo]
                        break
                if best:
                    break
            if best is None:
                best = waits
            extra = best[1:]
            if len(extra) > len(donors):
                unresolved.append(inst)
                continue
            for w in extra:
                d = donors.pop()
                dsi = d.sync_info
                dsi.on_wait = [w]
                d.sync_info = dsi
            si.on_wait = best[:1]
            inst.sync_info = si
    if unresolved:
        import logging

        logging.warning(
            "_transitive_prune_waits: %d instructions still multi-wait: %s",
            len(unresolved),
            [i.name for i in unresolved[:10]],
        )


_NC = None


def _get_nc():
    global _NC
    if _NC is None:
        _NC = _build()
        _transitive_prune_waits(_NC)
        # Populate .instr bytes for InstISA subclasses (custom-DVE ops).
        # Raw Bass doesn't run this pass; without it the NEFF compiler sees
        # empty .instr -> "ISA wrong length". Must run AFTER wait pruning:
        # the encoder bakes sync_info in and asserts <=1 wait per InstISA.
        from concourse.library_overlay import lower_extended_insts

        lower_extended_insts(_NC)
    return _NC


def kernel(x):
    global _LAST_EXEC_NS, _LAST_RESULT
    x = np.asarray(x, dtype=np.float32)
    assert x.shape == (B, C, T), x.shape
    flat = x.reshape(ROWS, T)
    # Bake the stencil clamp duplicates in on the host: pad[:, m] = x[m-1]
    # with x[-1] := x[0] and x[T] := x[T-1].
    pad = np.empty((ROWS, T + 2), dtype=np.float32)
    pad[:, 1 : T + 1] = flat
    pad[:, 0] = flat[:, 0]
    pad[:, T + 1] = flat[:, T - 1]
    in_maps = [
        {"x": np.ascontiguousarray(pad[i * RPC : (i + 1) * RPC])}
        for i in range(N_CORES)
    ]
    nc = _get_nc()
    res = run_bass_kernel_spmd(
        nc,
        in_maps,
        core_ids=list(range(N_CORES)),
        trace=os.environ.get("K_TRACE", "0") == "1",
    )
    _LAST_RESULT = res
    _LAST_EXEC_NS = res.exec_time_ns
    out = np.concatenate([r["out"] for r in res.results], axis=0)
    return np.ascontiguousarray(out.reshape(B, C, T))



# revision 21
# speedup vs baseline: 1.1622x; 1.0116x over previous
"""Trainium2 Bass kernel for nn_Activation1d (upsample2x-linear -> SiLU -> downsample2x).

Math: with align_corners=False linear resize, UP_RATIO=2, the whole op reduces
to a 3-tap stencil along T:
    a[j] = 0.25*(3*x[j] + x[j-1])      (x[-1] clamped to x[0])
    b[j] = 0.25*(3*x[j] + x[j+1])      (x[T] clamped to x[T-1])
    out[j] = 0.5*(silu(a[j]) + silu(b[j]))

Pure pointwise over (B, C): shard B*C = 8192 rows across 8 cores, T stays local.
"""

import os
import sys
from contextlib import ExitStack

import numpy as np

for _p in ("/opt/trn_rl_repo",):
    if _p not in sys.path:
        sys.path.insert(0, _p)

import bass_rust
import concourse.bass as bass
import concourse.mybir as mybir
from concourse import tile
from concourse.bass_utils import run_bass_kernel_spmd

N_CORES = 8
B, C, T = 16, 512, 8192
ROWS = B * C                 # 8192
RPC = ROWS // N_CORES        # 1024 rows per core
P = 128                      # SBUF partitions
N_RT = RPC // P              # 8 row-tiles per core
W = 2048                     # free-dim compute chunk width
NCH = T // W                 # chunks per row-tile

ALU = mybir.AluOpType
AFT = mybir.ActivationFunctionType
F32 = mybir.dt.float32

# --- tunables (env-overridable for experiments) ---
CDT_NAME = os.environ.get("K_CDT", "float16")       # compute dtype for DVE ops
USE_STT = os.environ.get("K_STT", "1") == "1"        # scalar_tensor_tensor vs (t3 + add)
# Inputs via SWDGE (gpsimd); outputs MUST be HWDGE (sync): compute
# instructions waiting on an SWDGE out-DMA's lane semaphore hang the device
# (the +16 never lands), while the same WAR pattern on DMAHW lanes works.
OUT_DMA_ENGINE = os.environ.get("K_ODMA", "sync")
IN_DMA_ENGINE = os.environ.get("K_IDMA", "gpsimd")

_LAST_EXEC_NS = None
_LAST_RESULT = None


def _build():
    cdt = getattr(mybir.dt, CDT_NAME)
    # Tile's stale SBUF cap (192K) leaves real capacity (208K usable) unused;
    # this kernel needs ~197K per partition.
    import concourse.tile_utils as _tu

    _tu.max_sbuf_usage = 208 * 1024
    nc = bass.Bass()
    # Input arrives host-padded to T+2 columns: x_pad[:, 0] = x[:, 0],
    # x_pad[:, 1:T+1] = x, x_pad[:, T+1] = x[:, T-1]. The clamp duplicates
    # are baked in on the host so no SBUF edge fills are needed and every
    # +-1-shifted stencil read is an aligned f32 view.
    x_ext = nc.declare_dram_parameter("x", [RPC, T + 2], F32, isOutput=False)
    o_ext = nc.declare_dram_parameter("out", [RPC, T], F32, isOutput=True)

    # Back-half finisher assignment: LN tiles do oc = 0.5*(sa+sb) in one
    # custom-DVE op (8.8 us, all DVE); mul tiles split it as TT-add on DVE
    # (4.4 us, into tb so the ACT WAR lands on TTb's free wait slot) +
    # Copy-with-scale on ACT (7.1 us). 4/4 balances DVE ~= ACT ~= 142 us,
    # under the ~160 us DMA floor.
    ln_tiles = {0, 2, 4, 6}

    with tile.TileContext(nc) as tc:
        with ExitStack() as ctx:
            xpool = ctx.enter_context(tc.tile_pool(name="xp", bufs=4))
            tpool = ctx.enter_context(tc.tile_pool(name="tp", bufs=2))
            opool = ctx.enter_context(tc.tile_pool(name="op", bufs=2))

            in_dma = getattr(nc, IN_DMA_ENGINE)
            out_dma = getattr(nc, OUT_DMA_ENGINE)

            # [P,1] scalar-slot constants for ln_bwd_dx (the TTSS scale
            # slots want DATA_PTR APs; float immediates mis-encode).
            cpool = ctx.enter_context(tc.tile_pool(name="cp", bufs=1))
            c0 = cpool.tile([P, 1], F32, tag="c0")
            cm1 = cpool.tile([P, 1], F32, tag="cm1")
            nc.vector.memset(c0[:], 0.0)
            nc.vector.memset(cm1[:], -1.0)

            # DMA budget: broken DGE lane-reuse in this stack means at most 8
            # DMAs per ring (SWDGE qPoolDynamic / HWDGE qSPDynamicHW) so no
            # lane is ever reused: 8 full-row loads (SWDGE) + 8 full-row
            # stores (sync HWDGE). The in-DMA casts f32->f16 in flight
            # (SWDGE-only feature): SBUF holds only f16, so 4 xs bufs fit
            # and the ~25 us load latency comes off the critical path.
            #
            # Software pipeline, back half shifted one tile so neither
            # engine waits mid-tile on the other's fresh output; every
            # instruction carries at most one semaphore wait (walrus
            # encodes only one).
            live = {}
            fin_names = {}
            in_dma_names = []
            for r in range(N_RT + 1):
                if r < N_RT:
                    rows = slice(r * P, (r + 1) * P)
                    xs = xpool.tile([P, T + 2], cdt, tag="xs")
                    din = in_dma.dma_start(xs[:], x_ext[rows, :])
                    in_dma_names.append(din.ins.name)
                    # ta <- 3*x[j] (misaligned f16 read runs in 2x mode);
                    # tb <- 3*x[j] + x[j+1]; then ta += x[j-1] in place.
                    # TS carries the DMA-lane wait (its ta WAR is program-
                    # order via the DVE finisher two tiles back); TTb carries
                    # the tb WAR (ACT mul r-2) in its free wait slot.
                    ta = tpool.tile([P, T], cdt, tag="ta")
                    tb = tpool.tile([P, T], cdt, tag="tb")
                    nc.vector.tensor_scalar_mul(ta[:], xs[:, 1 : T + 1], 3.0)
                    nc.vector.tensor_add(tb[:], ta[:], xs[:, 2 : T + 2])
                    nc.vector.tensor_add(ta[:], ta[:], xs[:, 0:T])
                    # silu in place, 0.25 folded into ACT's input scale;
                    # silu_b first so it overlaps the in-place ta add.
                    nc.scalar.activation(tb[:], tb[:], AFT.Silu, scale=0.25)
                    nc.scalar.activation(ta[:], ta[:], AFT.Silu, scale=0.25)
                if r >= 1:
                    ta_p, tb_p, oc_p = live.pop(r - 1)
                    prows = slice((r - 1) * P, r * P)
                    if (r - 1) in ln_tiles:
                        # oc = (ta - tb*(-1) - 0)*0.5, f16 -> f32, one DVE op
                        fin = nc.vector.ln_bwd_dx(
                            oc_p[:], ta_p[:], tb_p[:], cm1[:], c0[:], scale=0.5
                        )
                    else:
                        # sum into tb so ta's last reader stays on DVE and
                        # tb's WAR lands on TTb (which has a free wait slot)
                        fin = nc.vector.tensor_add(tb_p[:], ta_p[:], tb_p[:])
                        nc.scalar.mul(oc_p[:], tb_p[:], 0.5)
                    fin_names[r - 1] = fin.ins.name
                    out_dma.dma_start(o_ext[prows, :], oc_p[:])
                if r < N_RT:
                    # Dummy first-writer claim of oc on DVE: absorbs the WAR
                    # wait on the out-DMA lane (r-2) so the finisher carries
                    # only its RAW wait. Emitted AFTER the back half so a
                    # late out-DMA cannot head-of-line-block the finisher.
                    oc = opool.tile([P, T], F32, tag="oc")
                    nc.vector.memset(oc[:, 0:1], 0.0)
                    live[r] = (ta, tb, oc)
    nc._pacing_meta = (fin_names, in_dma_names)
    return nc


_PRUNABLE = (
    bass_rust.InstDMACopy,
    bass_rust.InstTensorCopy,
    bass_rust.InstTensorTensor,
    bass_rust.InstTensorScalarPtr,
    bass_rust.InstActivation,
    bass_rust.InstCustomDveAnt,
)


def _transitive_prune_waits(nc):
    """Reduce every prunable instruction to at most one semaphore wait.

    This walrus build's engine/DMA ISA structs hold a single sync wait per
    instruction, but Tile's scheduler emits one wait per dependent proc
    because its vector clock is not transitively minimal across procs.

    Phase 1 simulates the emitted program (greedy topological execution over
    per-engine in-order streams), recording for every semaphore value the
    happens-before knowledge it implies and a global feasible order.
    Phase 2 drops waits implied by program order + remaining waits; if more
    than one wait survives, it strengthens one wait (raising its threshold
    to a value already reached earlier in the phase-1 order, so no cycle can
    form) until that single wait implies all the others.

    Soundness: engines complete instructions in stream order (DVE/ACT/SP);
    per-lane DMA updates land in issue order (Tile serializes lane reuse);
    Pool compute may complete out of order across Q7 cores, so no transitive
    knowledge is propagated through the Pool semaphore.
    """
    f = nc.m.functions[0]
    streams = {}
    for b in f.blocks:
        for inst in b.instructions:
            streams.setdefault(str(inst.engine), []).append(inst)

    def merge(dst, src):
        for s, v in src.items():
            if dst.get(s, 0) < v:
                dst[s] = v

    # ---- phase 1: simulate, collect logs ----
    sem_val = {}
    sem_log = {}        # sem -> list of (cum_value, knowledge, step)
    proc_know = {e: {} for e in streams}
    proc_self = {e: {} for e in streams}
    ptr = {e: 0 for e in streams}
    inst_info = {}      # id(inst) -> (base knowledge, step)
    step = 0

    def knowledge_of(sem, val, max_step=None):
        k = {sem: val}
        if sem.startswith("Pool"):
            return k
        for cum, kn, st in sem_log.get(sem, ()):
            if max_step is not None and st >= max_step:
                break
            merge(k, kn)
            if cum >= val:
                break
        return k

    def satisfied(w):
        v = sem_val.get(w.ant_name, 0)
        return v == w.wait_value if w.wait_mode == "sem-eq-imm" else v >= w.wait_value

    def execute(eng, inst):
        nonlocal step, done
        si = inst.sync_info
        waits = list(si.on_wait) if si is not None else []
        base = dict(proc_know[eng])
        merge(base, proc_self[eng])
        inst_info[id(inst)] = (dict(base), step)
        acc = base
        for w in waits:
            merge(acc, knowledge_of(w.ant_name, w.wait_value))
        proc_know[eng] = acc
        is_dma = isinstance(inst, bass_rust.InstDMACopy)
        if si is not None:
            for u in si.on_update:
                s = u.ant_name
                dv = {
                    "sem-add-imm": u.update_value,
                    "sem-inc": 1,
                    "sem-dec": -1,
                    "sem-sub-imm": -u.update_value,
                }[u.update_mode]
                nv = sem_val.get(s, 0) + dv
                sem_val[s] = nv
                kn = dict(proc_know[eng])
                merge(kn, proc_self[eng])
                if not is_dma and eng != "EngineType.Pool":
                    # Pool (8 Q7 cores) completes out of order: a later Pool
                    # instruction cannot assume earlier ones finished.
                    proc_self[eng][s] = max(proc_self[eng].get(s, 0), nv)
                kn[s] = nv
                sem_log.setdefault(s, []).append((nv, kn, step))
        ptr[eng] += 1
        done += 1
        step += 1

    total = sum(len(s) for s in streams.values())
    done, progress = 0, True
    while done < total and progress:
        progress = False
        # Execute DMAs as late as possible so compute events order before
        # them in the recorded feasible order (maximizes strengthening).
        for eng, stream in streams.items():
            while ptr[eng] < len(stream):
                inst = stream[ptr[eng]]
                si = inst.sync_info
                waits = list(si.on_wait) if si is not None else []
                if isinstance(inst, bass_rust.InstDMACopy):
                    break
                if not all(satisfied(w) for w in waits):
                    break
                execute(eng, inst)
                progress = True
        if progress:
            continue
        for eng, stream in streams.items():
            if ptr[eng] < len(stream):
                inst = stream[ptr[eng]]
                si = inst.sync_info
                waits = list(si.on_wait) if si is not None else []
                if isinstance(inst, bass_rust.InstDMACopy) and all(
                    satisfied(w) for w in waits
                ):
                    execute(eng, inst)
                    progress = True
                    break
    if done < total:
        import logging

        logging.warning(
            "_transitive_prune_waits: simulation stalled at %d/%d; "
            "no pruning applied",
            done,
            total,
        )
        return

    # ---- phase 2: prune / strengthen ----
    remaining_multi = []
    for eng, stream in streams.items():
        for inst in stream:
            si = inst.sync_info
            waits = list(si.on_wait) if si is not None else []
            if len(waits) < 2:
                continue
            if not isinstance(inst, _PRUNABLE) or any(
                w.wait_mode != "sem-ge-imm" for w in waits
            ):
                remaining_multi.append(inst)
                continue
            base, my_step = inst_info[id(inst)]

            def implied(k, ws):
                return all(k.get(w.ant_name, 0) >= w.wait_value for w in ws)

            # A DMA's wait on its own update lane (Tile's lane-reuse
            # throttle) is load-bearing for the DGE hardware beyond its
            # ordering semantics: dropping it wedges the device even when
            # the ordering is transitively guaranteed. Never touch those.
            own_lanes = set()
            if isinstance(inst, bass_rust.InstDMACopy) and si is not None:
                own_lanes = {u.ant_name for u in si.on_update}
            fixed = [w for w in waits if w.ant_name in own_lanes]
            # 1) drop waits implied by base + the other waits (greedy, all orders)
            import itertools

            best = None
            for order in itertools.permutations(range(len(waits))):
                a = dict(base)
                for w in fixed:
                    merge(a, knowledge_of(w.ant_name, w.wait_value))
                kp = [i for i in range(len(waits)) if waits[i] in fixed]
                for i in order:
                    w = waits[i]
                    if w in fixed:
                        continue
                    if a.get(w.ant_name, 0) >= w.wait_value:
                        continue
                    kp.append(i)
                    merge(a, knowledge_of(w.ant_name, w.wait_value))
                if best is None or len(kp) < len(best):
                    best = kp
                if len(kp) <= 1:
                    break
            kept = [waits[i] for i in sorted(best)]
            # 2) strengthen: find one sem whose (possibly later) value implies all
            if len(kept) > 1 and fixed:
                remaining_multi.append(inst)
                continue
            if len(kept) > 1:
                chosen = None
                cands = sorted(
                    {w.ant_name for w in waits},
                    key=lambda s: (s.startswith("DMA"), s),
                )
                for s in cands:
                    if s.startswith("Pool"):
                        continue
                    k = dict(base)
                    for cum, kn, st in sem_log.get(s, ()):
                        if st >= my_step:
                            break  # only events already ordered before us
                        merge(k, kn)
                        k[s] = max(k.get(s, 0), cum)
                        if implied(k, waits):
                            chosen = (s, cum)
                            break
                    if chosen:
                        break
                if chosen:
                    tmpl = next(w for w in waits if w.ant_name == chosen[0])
                    tmpl.wait_value = chosen[1]
                    kept = [tmpl]
                else:
                    remaining_multi.append(inst)
                    continue
            if len(kept) != len(waits) or any(
                k.wait_value != w.wait_value for k, w in zip(kept, waits)
            ):
                si.on_wait = kept
                inst.sync_info = si
    # ---- phase 3: non-prunable multi-wait instructions (the tail drain) ----
    # Reduce to the minimal wait subset via transitivity, keep one wait, and
    # move the rest onto zero-wait tail instructions (event semaphores) that
    # execute before NEFF completion. Sound: the conditions depend only on
    # DMAs issued in the main region, so no donor can deadlock, and every
    # stream must finish before the NEFF signals done.
    import itertools as _it

    unresolved = []
    if remaining_multi:
        last_dma_step = max(
            (inst_info[id(i)][1] for s in streams.values() for i in s
             if isinstance(i, bass_rust.InstDMACopy) and id(i) in inst_info),
            default=0,
        )
        donors = [
            i
            for s in streams.values()
            for i in s
            if isinstance(
                i, (bass_rust.InstEventSemaphore, bass_rust.InstDrain)
            )
            and i.sync_info is not None
            and not list(i.sync_info.on_wait)
            and inst_info.get(id(i), (None, -1))[1] > last_dma_step
        ]
        for inst in remaining_multi:
            si = inst.sync_info
            waits = list(si.on_wait)
            if any(w.wait_mode != "sem-ge-imm" for w in waits):
                unresolved.append(inst)
                continue
            base, _st = inst_info[id(inst)]
            best = None
            for r in range(1, len(waits) + 1):
                for combo in _it.combinations(range(len(waits)), r):
                    k = dict(base)
                    for i in combo:
                        merge(k, knowledge_of(waits[i].ant_name, waits[i].wait_value))
                    if all(k.get(w.ant_name, 0) >= w.wait_value for w in waits):
                        best = [waits[i] for i in combo]
                        break
                if best:
                    break
            if best is None:
                best = waits
            extra = best[1:]
            if len(extra) > len(donors):
                unresolved.append(inst)
                continue
            for w in extra:
                d = donors.pop()
                dsi = d.sync_info
                dsi.on_wait = [w]
                d.sync_info = dsi
            si.on_wait = best[:1]
            inst.sync_info = si
    if unresolved:
        import logging

        logging.warning(
            "_transitive_prune_waits: %d instructions still multi-wait: %s",
            len(unresolved),
            [i.name for i in unresolved[:10]],
        )


_NC = None


def _pace_in_dmas(nc):
    """Delay in-DMA issue so loads and stores alternate at the DMA engine
    pool. With 4 prefetched loads plus WAR-gated follow-ons, 6 loads queue
    ahead of the first store; every store then drains ~30 us late and the
    oc WAR passes that delay to the finishers. Strengthen in-DMA(r>=4)'s
    existing DVE WAR wait to 'finisher(r-3) done' (strictly later on the
    same independent stream -> still sound, no cycles): one load issues per
    iteration, right as one store issues."""
    fin_names, in_dma_names = nc._pacing_meta
    f = nc.m.functions[0]
    dve = [
        i
        for b in f.blocks
        for i in b.instructions
        if str(i.engine) == "EngineType.DVE"
    ]
    cum = {}
    c = 0
    for inst in dve:
        si = inst.sync_info
        if si is not None:
            for u in si.on_update:
                if u.update_mode in ("sem-inc", "sem-add-imm"):
                    c += 1 if u.update_mode == "sem-inc" else u.update_value
        cum[inst.name] = c
    by_name = {i.name: i for b in f.blocks for i in b.instructions}
    for r in range(4, len(in_dma_names)):
        tgt = cum.get(fin_names.get(r - 3, ""), None)
        dma = by_name.get(in_dma_names[r])
        if tgt is None or dma is None or dma.sync_info is None:
            continue
        si = dma.sync_info
        waits = list(si.on_wait)
        for w in waits:
            if w.ant_name.startswith("DVE") and w.wait_mode == "sem-ge-imm":
                if tgt > w.wait_value:
                    w.wait_value = tgt
        si.on_wait = waits
        dma.sync_info = si


def _get_nc():
    global _NC
    if _NC is None:
        _NC = _build()
        _transitive_prune_waits(_NC)
        _pace_in_dmas(_NC)
        # Populate .instr bytes for InstISA subclasses (custom-DVE ops).
        # Raw Bass doesn't run this pass; without it the NEFF compiler sees
        # empty .instr -> "ISA wrong length". Must run AFTER wait pruning:
        # the encoder bakes sync_info in and asserts <=1 wait per InstISA.
        from concourse.library_overlay import lower_extended_insts

        lower_extended_insts(_NC)
    return _NC


def kernel(x):
    global _LAST_EXEC_NS, _LAST_RESULT
    x = np.asarray(x, dtype=np.float32)
    assert x.shape == (B, C, T), x.shape
    flat = x.reshape(ROWS, T)
    # Bake the stencil clamp duplicates in on the host: pad[:, m] = x[m-1]
    # with x[-1] := x[0] and x[T] := x[T-1].
    pad = np.empty((ROWS, T + 2), dtype=np.float32)
    pad[:, 1 : T + 1] = flat
    pad[:, 0] = flat[:, 0]
    pad[:, T + 1] = flat[:, T - 1]
    in_maps = [
        {"x": np.ascontiguousarray(pad[i * RPC : (i + 1) * RPC])}
        for i in range(N_CORES)
    ]
    nc = _get_nc()
    res = run_bass_kernel_spmd(
        nc,
        in_maps,
        core_ids=list(range(N_CORES)),
        trace=os.environ.get("K_TRACE", "0") == "1",
    )
    _LAST_RESULT = res
    _LAST_EXEC_NS = res.exec_time_ns
    out = np.concatenate([r["out"] for r in res.results], axis=0)
    return np.ascontiguousarray(out.reshape(B, C, T))

